# revision 16
# baseline (speedup 1.0000x reference)
"""Trainium2 Bass kernel for nn_EncodingModule2d (vq_codebook).

Pipeline per batch item (pure data parallel, 1 item per NeuronCore, 8 cores):
  stem:   s = conv_w @ x  (1x1 conv as 256x256 matmul over 4096 positions)
          y = relu(BN2(s))                          -- BN folded into weights on host
  vq:     dist2[n,k] = |y_n|^2 - 2<y_n, c_k> + |c_k|^2
          a = softmax_k(scales_k * dist2)
          agg[k,:] = sum_n a[n,k] (y_n - c_k)
  post:   z = mean_k relu(BN1(agg))                 -- BN folded on host
          g = sigmoid(head_w @ z + head_b)
  out:    relu(x + x * g) = relu(x * (1 + g))

dtype strategy: bf16 for x, conv weights, y and the softmax/aggregation
path (halves input DMA, fast weight loads, 2x DVE modes); fp16 for the
scales and squares feeding the s_k*|y_n|^2 logit term (a bf16-rounded
scale biases every position's logit for that center coherently and does
NOT average out over n: 5e-2 rel err in bf16, 5e-3 in fp16); fp32 PSUM
accumulation, fp32 tail math and output.

Engine placement: the PE runs only the 4 real matmul families (stem,
distance, aggregation, head). Both transpose families ((d,n)->(n,d) for
the aggregation operand, (k,n)->(n,k) for the softmax logits) run on the
DMA XBAR transpose engine, issued from the sync sequencer, which is
otherwise idle; XBAR occupies the issuing engine for ~1us/256KB so it
must not share the scalar (ACT) queue. Row-sums of the softmax weights
(needed for the -sum_a*c correction) come from a gpsimd partition-reduce
instead of a ones-column matmul. Squares split gpsimd (block 0) / DVE
(block 1).
"""

import os
import sys

for _p in ("/opt/trn_rl_repo",):
    if _p not in sys.path and os.path.isdir(_p):
        sys.path.insert(0, _p)

from contextlib import ExitStack

import numpy as np

import concourse.bass as bass
import concourse.tile as tile
from concourse import bacc, bass_isa, mybir
from concourse.bass_utils import run_bass_kernel_spmd
from concourse.masks import make_identity

F32 = mybir.dt.float32
F16 = mybir.dt.float16
BF16 = mybir.dt.bfloat16
AF = mybir.ActivationFunctionType
ALU = mybir.AluOpType

B, D, H, W, K = 8, 256, 64, 64, 32
HW = H * W          # 4096 spatial positions
NB = D // 128       # 2 channel blocks of 128
NS = HW // 512      # 8 n-slices of 512
NCH = HW // 128     # 32 n-chunks of 128
EPS = 1e-5
N_CORES = 8

# pkb (bf16) per-c-block column layout: [wT | ct2 | ones | pad | hwT]
PKB_W = 0
PKB_CT2 = D
PKB_ONE = D + K
PKB_HW = D + K + 2
PKB_COLS = 2 * D + K + 2
CW = D + 1          # y_nd chunk width: 256 y + ones column


def _strided_cols(t, start, step, count, width):
    """AP over columns [start + i*step : start + i*step + width) of a 2D tile."""
    a = t[:, start : start + 1]
    return bass.AP(tensor=a.tensor, offset=a.offset, ap=[a.ap[0], [step, count], [1, width]])


def _build_program(has_bias2):
    nc = bacc.Bacc("TRN2", target_bir_lowering=False, debug=False, num_devices=N_CORES)

    x_d = nc.dram_tensor("x", [D, HW], BF16, kind="ExternalInput").ap()
    # host-packed in SBUF layout: [128 rows, c * cols] (contiguous rows)
    pkb_d = nc.dram_tensor("pkb", [128, NB * PKB_COLS], BF16, kind="ExternalInput").ap()
    pkf_d = nc.dram_tensor("pkf", [128, NB * 4], F32, kind="ExternalInput").ap()
    pks_d = nc.dram_tensor("pks", [K, D + 2], F32, kind="ExternalInput").ap()
    srf_d = nc.dram_tensor("srf", [1, K], F16, kind="ExternalInput").ap()
    out_d = nc.dram_tensor("out", [D, HW], F32, kind="ExternalOutput").ap()

    with tile.TileContext(nc) as tc, ExitStack() as ctx:
        sb = ctx.enter_context(tc.tile_pool(name="sb", bufs=1))

        x_sb = sb.tile([128, NB, HW], BF16)
        pkb = sb.tile([128, NB, PKB_COLS], BF16)
        pkf = sb.tile([128, NB, 4], F32)
        pks = sb.tile([K, D + 2], F32)
        srep = sb.tile([128, K], F16)

        # ---- loads: consts, then x pieces across 4 DMA rings ----------
        qeng = [nc.sync, nc.scalar]
        nc.sync.dma_start(pkb[:], pkb_d)
        nc.scalar.dma_start(pkf[:], pkf_d)
        nc.scalar.dma_start(srep[:], srf_d.partition_broadcast(128))
        nc.scalar.dma_start(pks[:], pks_d)
        xq = [[nc.sync, nc.gpsimd, nc.sync, nc.sync],
              [nc.scalar, nc.gpsimd, nc.scalar, nc.scalar]]
        pieces = [(0, 1024), (1024, 2048), (2048, 3072), (3072, 4096)]
        for q, (lo, hi) in enumerate(pieces):
            cs = slice(lo, hi)
            for c in range(NB):
                xq[c][q].dma_start(x_sb[:, c, cs], x_d[c * 128 : (c + 1) * 128, cs])

        wT = pkb[:, :, PKB_W : PKB_W + D]              # wT[c,:,o] per c-block
        ct2 = pkb[:, :, PKB_CT2 : PKB_CT2 + K]         # -2*scales[k]*centers[k,d]
        onecol = pkb[:, 0, PKB_ONE : PKB_ONE + 1]      # bf16 ones column
        hwT = pkb[:, :, PKB_HW : PKB_HW + D]           # head_w.T / K (bf16)
        chv = pkf[:, :, 0:4]                           # [bias2, s1, bb1, head_b]
        ckd = pks[:, 0:D]                              # centers (k,d)
        sc2col = pks[:, D : D + 1]                     # scales[k]*|c_k|^2 (bias column)

        identb = sb.tile([32, 32], BF16)
        make_identity(nc, identb[:])
        ident128 = sb.tile([128, 128], BF16)
        make_identity(nc, ident128[:])
        identf = sb.tile([32, 32], F32)
        make_identity(nc, identf[:])

        # warm the exp table on ACT early (hidden under the x DMA)
        warm = sb.tile([128, 1], F32)
        nc.vector.memset(warm[:], 0.0)
        nc.scalar.activation(warm[:], warm[:], AF.Exp)

        wmt = sb.tile([128, 128], BF16)
        nc.gpsimd.memset(wmt[:], 0.0)

        # ---- big intermediates ----------------------------------------
        y_dn = sb.tile([128, NB, HW], BF16)      # relu(W'x): d on partitions
        y_nd = sb.tile([128, NCH * CW], BF16)    # per chunk: 256 y cols + ones
        ysq = sb.tile([128, NB, HW], F16)        # y_dn^2 (fp16: exact-ish)
        lkn = sb.tile([32, HW], BF16)            # logits in (k, n) layout
        lnk = sb.tile([128, NCH * K], BF16)      # logits in (n, k) layout
        esub = sb.tile([128, NCH * K], BF16)     # logits - max
        e_sb = sb.tile([128, NCH * K], BF16)     # exp(...)
        a_sb = sb.tile([128, NCH * K], BF16)     # softmax weights
        out_sb = sb.tile([128, NB, HW], F32)

        # ones columns of y_nd
        nc.vector.tensor_copy(
            _strided_cols(y_nd, D, CW, NCH, 1),
            onecol.rearrange("p (u k) -> p u k", u=1).broadcast_to((128, NCH, 1)))

        with ExitStack() as outer_ctx:
            psG = outer_ctx.enter_context(tc.tile_pool(name="psG", bufs=1, space="PSUM"))
            pagg = psG.tile([32, CW], F32, name="pagg")

            stem_ctx = outer_ctx.enter_context(ExitStack())
            psB = stem_ctx.enter_context(tc.tile_pool(name="psB", bufs=3, space="PSUM"))
            psA = stem_ctx.enter_context(tc.tile_pool(name="psA", bufs=2, space="PSUM"))
            psK = stem_ctx.enter_context(tc.tile_pool(name="psK", bufs=2, space="PSUM"))

            maxt = sb.tile([128, NCH], BF16)
            sumt = sb.tile([128, NCH], F32)
            rcp = sb.tile([128, NCH], F32)
            rcpb = sb.tile([128, NCH], BF16)

            # HAM warm-up: dummy transposes with no DMA dependency keep the
            # PE clock gate at 8/8 until the first x piece lands.
            for i in range(20):
                pWr = psA.tile([128, 160], BF16, name="warm", tag="pA")
                nc.tensor.transpose(pWr[0:32, 128:160], identb[:], identb[:])
                if i < 12:
                    nc.tensor.transpose(pWr[:, 0:128], wmt[:], ident128[:])

            def emit_softmax(g0, gn):
                gs = slice(g0, g0 + gn)
                cs = slice(g0 * K, (g0 + gn) * K)
                lp3 = lnk[:, cs].rearrange("p (g k) -> p g k", g=gn)
                nc.vector.tensor_reduce(out=maxt[:, gs], in_=lp3,
                                        axis=mybir.AxisListType.X, op=ALU.max)
                mb = maxt[:, gs].rearrange("p (g u) -> p g u", u=1).broadcast_to((128, gn, K))
                nc.vector.tensor_tensor(
                    out=esub[:, cs].rearrange("p (g k) -> p g k", g=gn),
                    in0=lp3, in1=mb, op=ALU.subtract)
                nc.scalar.activation(e_sb[:, cs], esub[:, cs], AF.Exp)
                nc.vector.tensor_reduce(out=sumt[:, gs],
                                        in_=e_sb[:, cs].rearrange("p (g k) -> p g k", g=gn),
                                        axis=mybir.AxisListType.X, op=ALU.add)
                nc.vector.reciprocal(rcp[:, gs], sumt[:, gs])
                nc.vector.tensor_copy(rcpb[:, gs], rcp[:, gs])
                rb = rcpb[:, gs].rearrange("p (g u) -> p g u", u=1).broadcast_to((128, gn, K))
                nc.vector.tensor_tensor(out=a_sb[:, cs].rearrange("p (g k) -> p g k", g=gn),
                                        in0=e_sb[:, cs].rearrange("p (g k) -> p g k", g=gn),
                                        in1=rb, op=ALU.mult)
                # aggregation for the finished chunks, accumulated in PSUM
                for g in range(g0, g0 + gn):
                    nc.tensor.matmul(
                        pagg[:],
                        a_sb[:, g * K : (g + 1) * K],
                        y_nd[:, g * CW : (g + 1) * CW],
                        start=(g == 0), stop=(g == NCH - 1))

            for s in range(NS):
                ns = slice(s * 512, (s + 1) * 512)
                # --- stem B: y_dn[o, ns] = relu(sum_c wT[c,o]x[c,ns] + bias2[o])
                for o in range(NB):
                    pB = psB.tile([128, 512], F32)
                    for c in range(NB):
                        nc.tensor.matmul(
                            pB[:],
                            wT[:, c, o * 128 : (o + 1) * 128],
                            x_sb[:, c, ns],
                            start=(c == 0),
                            stop=(c == NB - 1),
                        )
                    dst = y_dn[:, o, ns]
                    if s % 2 == 0:
                        if has_bias2:
                            nc.scalar.activation(dst, pB[:], AF.Relu, bias=chv[:, o, 0:1])
                        else:
                            nc.scalar.activation(dst, pB[:], AF.Relu)
                    else:
                        if has_bias2:
                            nc.vector.tensor_scalar(
                                out=dst, in0=pB[:], scalar1=chv[:, o, 0:1],
                                scalar2=0.0, op0=ALU.add, op1=ALU.max)
                        else:
                            nc.vector.tensor_scalar_max(out=dst, in0=pB[:], scalar1=0.0)

                # --- stem A: y_nd chunk j via PE transpose of y_dn --------
                for half in range(2):
                    pA = psA.tile([128, 512], BF16)
                    j0 = 4 * s + 2 * half
                    for ci in range(2):
                        j = j0 + ci
                        jc = slice(j * 128, (j + 1) * 128)
                        for c in range(NB):
                            nc.tensor.transpose(
                                pA[:, (2 * ci + c) * 128 : (2 * ci + c + 1) * 128],
                                y_dn[:, c, jc], ident128[:])
                    dst = _strided_cols(y_nd, j0 * CW, CW, 2, D)
                    if half == 0:
                        nc.scalar.activation(dst, pA[:], AF.Identity)
                    else:
                        nc.vector.tensor_copy(dst, pA[:])

                # --- squares + logits; slice pairs through s=5, then
                #     per-slice for s=6,7 to shorten the serial tail --------
                if s in (1, 3, 5):
                    sls = (s - 1, s)
                elif s >= 6:
                    sls = (s,)
                else:
                    sls = ()
                if sls:
                    qs = slice(sls[0] * 512, (sls[-1] + 1) * 512)
                    if s >= 6:
                        nc.scalar.activation(ysq[:, 0, qs], y_dn[:, 0, qs], AF.Square)
                    else:
                        nc.gpsimd.tensor_mul(ysq[:, 0, qs], y_dn[:, 0, qs], y_dn[:, 0, qs])
                    nc.vector.tensor_tensor(out=ysq[:, 1, qs], in0=y_dn[:, 1, qs],
                                            in1=y_dn[:, 1, qs], op=ALU.mult)

                    for si, sl in enumerate(sls):
                        pK = psK.tile([32, 512], F32)
                        nsl = slice(sl * 512, (sl + 1) * 512)
                        nc.tensor.matmul(pK[:], ct2[:, 0, :], y_dn[:, 0, nsl],
                                         start=True, stop=False)
                        nc.tensor.matmul(pK[:], ct2[:, 1, :], y_dn[:, 1, nsl],
                                         start=False, stop=False)
                        nc.tensor.matmul(pK[:], srep[:], ysq[:, 0, nsl],
                                         start=False, stop=False)
                        nc.tensor.matmul(pK[:], srep[:], ysq[:, 1, nsl],
                                         start=False, stop=True)
                        dst = lkn[:, nsl]
                        if si == 0 and s != 7:
                            nc.scalar.activation(dst, pK[:], AF.Identity, bias=sc2col[:])
                        else:
                            nc.vector.tensor_scalar_add(out=dst, in0=pK[:],
                                                        scalar1=sc2col[:])
                    # logits (k,n) -> (n,k) via grouped DMA XBAR transpose
                    g0 = 4 * sls[0]
                    gt = 4 * len(sls)
                    nc.sync.dma_start(
                        lnk[:, g0 * K : (g0 + gt) * K].rearrange("p (t k) -> p t k", t=gt),
                        lkn[:, sls[0] * 512 : (sls[-1] + 1) * 512], transpose=True)

                # --- softmax + agg over finished logits groups ------------
                if s == 3:
                    emit_softmax(0, 16)
                elif s == 5:
                    emit_softmax(16, 8)
                elif s == 6:
                    emit_softmax(24, 4)
                elif s == 7:
                    emit_softmax(28, 4)

            # ---- tail: rowsum fold, BN1 + mean, head, gate --------------
            stem_ctx.close()
            psT = outer_ctx.enter_context(tc.tile_pool(name="psT", bufs=2, space="PSUM"))

            # agg[k,d] = pagg[k,d] - rowsum_a[k] * centers[k,d]
            rsc = sb.tile([32, D], F32)
            nc.vector.tensor_scalar_mul(out=rsc[:], in0=ckd[:], scalar1=pagg[:, D : D + 1])
            agg_sb = sb.tile([32, D], F32)
            nc.vector.tensor_tensor(out=agg_sb[:], in0=pagg[:, 0:D], in1=rsc[:], op=ALU.subtract)

            # BN1 + relu + mean over k  ->  z per d-block (ACT accumulates)
            z_t = sb.tile([128, NB], F32)
            t_sb = sb.tile([128, NB, K], F32)
            for b in range(NB):
                pT = psT.tile([128, 32], F32)
                nc.tensor.transpose(pT[:], agg_sb[:, b * 128 : (b + 1) * 128], identf[:])
                nc.scalar.activation(t_sb[:, b, :], pT[:], AF.Relu,
                                     bias=chv[:, b, 2:3], scale=chv[:, b, 1:2],
                                     accum_out=z_t[:, b : b + 1])

            # head: gate = 1 + sigmoid(head_w @ z + head_b)
            zb = sb.tile([128, NB], BF16)
            nc.vector.tensor_copy(zb[:], z_t[:])
            gate = sb.tile([128, NB], F32)
            eg = sb.tile([128, NB], F32)
            for o in range(NB):
                pH = psT.tile([128, 1], F32)
                for c in range(NB):
                    nc.tensor.matmul(pH[:], hwT[:, c, o * 128 : (o + 1) * 128],
                                     zb[:, c : c + 1],
                                     start=(c == 0), stop=(c == NB - 1))
                nc.scalar.activation(eg[:, o : o + 1], pH[:], AF.Sigmoid,
                                     bias=chv[:, o, 3:4])
            nc.vector.tensor_scalar_add(out=gate[:], in0=eg[:], scalar1=1.0)

            # gating: out = relu(x * gate[d]) ; stream out per 512-col block
            for hh in range(8):
                cs = slice(hh * 512, (hh + 1) * 512)
                for o in range(NB):
                    if (2 * hh + o) % 2 == 0:
                        nc.vector.tensor_scalar(out=out_sb[:, o, cs], in0=x_sb[:, o, cs],
                                                scalar1=gate[:, o : o + 1], scalar2=0.0,
                                                op0=ALU.mult, op1=ALU.max)
                    else:
                        nc.scalar.activation(out_sb[:, o, cs], x_sb[:, o, cs],
                                             AF.Relu, scale=gate[:, o : o + 1])
                    qeng[o].dma_start(out_d[o * 128 : (o + 1) * 128, cs], out_sb[:, o, cs])

    nc.compile()
    return nc


_PROGRAM_CACHE = {}


def _get_program(has_bias2):
    key = bool(has_bias2)
    if key not in _PROGRAM_CACHE:
        _PROGRAM_CACHE[key] = _build_program(key)
    return _PROGRAM_CACHE[key]


def _host_params(conv_w, bn2_g, bn2_b, bn2_m, bn2_v, centers, scales,
                 bn1_g, bn1_b, bn1_m, bn1_v, head_w, head_b):
    scale2 = bn2_g / np.sqrt(bn2_v + EPS)
    wT = (conv_w * scale2[:, None]).T.astype(np.float32)             # (c, o)
    bias2 = (bn2_b - bn2_m * scale2).astype(np.float32)
    ct2 = (-2.0 * scales[None, :] * centers.T).astype(np.float32)    # (d, k)
    c2 = (centers * centers).sum(axis=1)
    scc = (scales * c2).astype(np.float32)                           # (k,)
    s1 = bn1_g / np.sqrt(bn1_v + EPS)
    bb1 = bn1_b - bn1_m * s1
    chv = np.stack([bias2, s1.astype(np.float32), bb1.astype(np.float32),
                    head_b.astype(np.float32)], axis=1).astype(np.float32)  # (d, 4)
    hwT = (head_w.T / np.float32(K)).astype(np.float32)              # (d, o)

    # pkb packed per c-block in SBUF row layout [128, c*PKB_COLS]
    pkb = np.zeros((128, NB * PKB_COLS), np.float32)
    for c in range(NB):
        base = c * PKB_COLS
        pkb[:, base + PKB_W : base + PKB_W + D] = wT[c * 128 : (c + 1) * 128, :]
        pkb[:, base + PKB_CT2 : base + PKB_CT2 + K] = ct2[c * 128 : (c + 1) * 128, :]
        pkb[:, base + PKB_ONE] = 1.0
        pkb[:, base + PKB_HW : base + PKB_HW + D] = hwT[c * 128 : (c + 1) * 128, :]

    import ml_dtypes
    pkb16 = pkb.astype(ml_dtypes.bfloat16)

    pkf = np.zeros((128, NB * 4), np.float32)
    for c in range(NB):
        pkf[:, c * 4 : (c + 1) * 4] = chv[c * 128 : (c + 1) * 128, :]

    pks = np.zeros((K, D + 2), np.float32)
    pks[:, 0:D] = centers
    pks[:, D] = scc
    srf = scales.reshape(1, K).astype(np.float16)
    return pkb16, pkf, pks, srf, bias2


def _ensure_profile_hook():
    """Register the axon NTFF profile hook if the image lacks antenv.axon_hooks."""
    import types

    if "antenv.axon_hooks" in sys.modules:
        return
    try:
        import antenv

        mod = types.ModuleType("antenv.axon_hooks")
        _hook = [None]
        mod.set_axon_ntff_profile_hook = lambda h: _hook.__setitem__(0, h)
        mod.get_axon_ntff_profile_hook = lambda: _hook[0]
        sys.modules["antenv.axon_hooks"] = mod
        antenv.axon_hooks = mod
        from trn_agent_boot.trn_boot import _ntff_profile_via_ctypes

        mod.set_axon_ntff_profile_hook(
            _ntff_profile_via_ctypes("/opt/axon/libaxon_pjrt.so"))
        import concourse.bass_utils as _bu

        _bu.upload_artifacts = lambda d: d  # no artifact store in this container
    except Exception as e:  # profiling is best-effort
        print(f"profile hook setup failed: {e}", file=sys.stderr)


def kernel(x, conv_w, bn2_g, bn2_b, bn2_m, bn2_v, centers, scales,
           bn1_g, bn1_b, bn1_m, bn1_v, head_w, head_b):
    import ml_dtypes

    x = np.asarray(x, dtype=np.float32)
    xh = x.astype(ml_dtypes.bfloat16)
    pkb16, pkf, pks, srf, bias2 = _host_params(
        np.asarray(conv_w, np.float32), np.asarray(bn2_g, np.float32),
        np.asarray(bn2_b, np.float32), np.asarray(bn2_m, np.float32),
        np.asarray(bn2_v, np.float32), np.asarray(centers, np.float32),
        np.asarray(scales, np.float32), np.asarray(bn1_g, np.float32),
        np.asarray(bn1_b, np.float32), np.asarray(bn1_m, np.float32),
        np.asarray(bn1_v, np.float32), np.asarray(head_w, np.float32),
        np.asarray(head_b, np.float32))
    has_bias2 = bool(np.abs(bias2).max() > 0)
    nc = _get_program(has_bias2)

    shared = {"pkb": pkb16, "pkf": pkf, "pks": pks, "srf": srf}
    in_maps = [dict(shared, x=np.ascontiguousarray(xh[b].reshape(D, HW)))
               for b in range(N_CORES)]

    trace = bool(int(os.environ.get("KERNEL_TRACE", "0")))
    kwargs = {}
    if trace:
        _ensure_profile_hook()
        tdir = os.environ.get("KERNEL_TRACE_DIR")
        if tdir:
            os.makedirs(tdir, exist_ok=True)
            kwargs["tmpdir"] = tdir
    res = run_bass_kernel_spmd(nc, in_maps, list(range(N_CORES)), trace=trace, **kwargs)
    if trace:
        kernel.last_exec_time_ns = res.exec_time_ns
        kernel.last_results = res
    out = np.stack([res.results[b]["out"].reshape(D, H, W) for b in range(N_CORES)])
    return out.astype(np.float32)


# revision 18
# speedup vs baseline: 1.2349x; 1.2349x over previous
"""Trainium2 Bass kernel for nn_EncodingModule2d (vq_codebook).

Pipeline per batch item (pure data parallel, 1 item per NeuronCore, 8 cores):
  stem:   s = conv_w @ x  (1x1 conv as 256x256 matmul over 4096 positions)
          y = relu(BN2(s))                          -- BN folded into weights on host
  vq:     dist2[n,k] = |y_n|^2 - 2<y_n, c_k> + |c_k|^2
          a = softmax_k(scales_k * dist2)
          agg[k,:] = sum_n a[n,k] (y_n - c_k)
  post:   z = mean_k relu(BN1(agg))                 -- BN folded on host
          g = sigmoid(head_w @ z + head_b)
  out:    relu(x + x * g) = relu(x * (1 + g))

dtype strategy: bf16 for x, conv weights, y and the softmax/aggregation
path (halves input DMA, fast weight loads, 2x DVE modes); fp16 for the
scales and squares feeding the s_k*|y_n|^2 logit term (a bf16-rounded
scale biases every position's logit for that center coherently and does
NOT average out over n: 5e-2 rel err in bf16, 5e-3 in fp16); fp32 PSUM
accumulation, fp32 tail math and output.

Engine placement: the PE runs only the 4 real matmul families (stem,
distance, aggregation, head). Both transpose families ((d,n)->(n,d) for
the aggregation operand, (k,n)->(n,k) for the softmax logits) run on the
DMA XBAR transpose engine, issued from the sync sequencer, which is
otherwise idle; XBAR occupies the issuing engine for ~1us/256KB so it
must not share the scalar (ACT) queue. Row-sums of the softmax weights
(needed for the -sum_a*c correction) come from a gpsimd partition-reduce
instead of a ones-column matmul. Squares split gpsimd (block 0) / DVE
(block 1).
"""

import os
import sys

for _p in ("/opt/trn_rl_repo",):
    if _p not in sys.path and os.path.isdir(_p):
        sys.path.insert(0, _p)

from contextlib import ExitStack

import numpy as np

import concourse.bass as bass
import concourse.tile as tile
from concourse import bacc, bass_isa, mybir
from concourse.bass_utils import run_bass_kernel_spmd
from concourse.masks import make_identity

F32 = mybir.dt.float32
F16 = mybir.dt.float16
BF16 = mybir.dt.bfloat16
AF = mybir.ActivationFunctionType
ALU = mybir.AluOpType

B, D, H, W, K = 8, 256, 64, 64, 32
HW = H * W          # 4096 spatial positions
NB = D // 128       # 2 channel blocks of 128
NS = HW // 512      # 8 n-slices of 512
NCH = HW // 128     # 32 n-chunks of 128
EPS = 1e-5
N_CORES = 8

# pkb (bf16) per-c-block column layout: [wT | ct2 | ones | pad | hwT]
PKB_W = 0
PKB_CT2 = D
PKB_ONE = D + K
PKB_HW = D + K + 2
PKB_COLS = 2 * D + K + 2
CW = D + 1          # y_nd chunk width: 256 y + ones column


def _strided_cols(t, start, step, count, width):
    """AP over columns [start + i*step : start + i*step + width) of a 2D tile."""
    a = t[:, start : start + 1]
    return bass.AP(tensor=a.tensor, offset=a.offset, ap=[a.ap[0], [step, count], [1, width]])


def _build_program(has_bias2):
    nc = bacc.Bacc("TRN2", target_bir_lowering=False, debug=False, num_devices=N_CORES)

    x_d = nc.dram_tensor("x", [D, HW], BF16, kind="ExternalInput").ap()
    # host-packed in SBUF layout: [128 rows, c * cols] (contiguous rows)
    pkb_d = nc.dram_tensor("pkb", [128, NB * PKB_COLS], BF16, kind="ExternalInput").ap()
    pkf_d = nc.dram_tensor("pkf", [128, NB * 4], F32, kind="ExternalInput").ap()
    pks_d = nc.dram_tensor("pks", [K, D + 2], F32, kind="ExternalInput").ap()
    srf_d = nc.dram_tensor("srf", [1, K], F16, kind="ExternalInput").ap()
    out_d = nc.dram_tensor("out", [D, HW], F32, kind="ExternalOutput").ap()

    with tile.TileContext(nc) as tc, ExitStack() as ctx:
        sb = ctx.enter_context(tc.tile_pool(name="sb", bufs=1))

        x_sb = sb.tile([128, NB, HW], BF16)
        pkb = sb.tile([128, NB, PKB_COLS], BF16)
        pkf = sb.tile([128, NB, 4], F32)
        pks = sb.tile([K, D + 2], F32)
        srep = sb.tile([128, K], F16)

        # ---- loads: consts, then x pieces across 4 DMA rings ----------
        qeng = [nc.sync, nc.scalar]
        nc.sync.dma_start(pkb[:], pkb_d)
        nc.scalar.dma_start(pkf[:], pkf_d)
        nc.scalar.dma_start(srep[:], srf_d.partition_broadcast(128))
        nc.scalar.dma_start(pks[:], pks_d)
        pieces = [(0, 1024), (1024, 2048), (2048, 3072), (3072, 4096)]
        for q, (lo, hi) in enumerate(pieces):
            cs = slice(lo, hi)
            for c in range(NB):
                qeng[c].dma_start(x_sb[:, c, cs], x_d[c * 128 : (c + 1) * 128, cs])

        wT = pkb[:, :, PKB_W : PKB_W + D]              # wT[c,:,o] per c-block
        ct2 = pkb[:, :, PKB_CT2 : PKB_CT2 + K]         # -2*scales[k]*centers[k,d]
        onecol = pkb[:, 0, PKB_ONE : PKB_ONE + 1]      # bf16 ones column
        hwT = pkb[:, :, PKB_HW : PKB_HW + D]           # head_w.T / K (bf16)
        chv = pkf[:, :, 0:4]                           # [bias2, s1, bb1, head_b]
        ckd = pks[:, 0:D]                              # centers (k,d)
        sc2col = pks[:, D : D + 1]                     # scales[k]*|c_k|^2 (bias column)

        identb = sb.tile([32, 32], BF16)
        make_identity(nc, identb[:])
        ident128 = sb.tile([128, 128], BF16)
        make_identity(nc, ident128[:])
        identf = sb.tile([32, 32], F32)
        make_identity(nc, identf[:])

        # warm the exp table on ACT early (hidden under the x DMA)
        warm = sb.tile([128, 1], F32)
        nc.vector.memset(warm[:], 0.0)
        nc.scalar.activation(warm[:], warm[:], AF.Exp)

        wmt = sb.tile([128, 128], BF16)
        nc.gpsimd.memset(wmt[:], 0.0)

        # ---- big intermediates ----------------------------------------
        y_dn = sb.tile([128, NB, HW], BF16)      # relu(W'x): d on partitions
        y_nd = sb.tile([128, NCH * CW], BF16)    # per chunk: 256 y cols + ones
        ysq = sb.tile([128, NB, HW], F16)        # y_dn^2 (fp16: exact-ish)
        lkn = sb.tile([32, HW], BF16)            # logits in (k, n) layout
        lnk = sb.tile([128, NCH * K], BF16)      # logits in (n, k) layout
        esub = sb.tile([128, NCH * K], BF16)     # logits - max
        e_sb = sb.tile([128, NCH * K], BF16)     # exp(...)
        a_sb = sb.tile([128, NCH * K], BF16)     # softmax weights
        out_sb = sb.tile([128, NB, HW], F32)

        # ones columns of y_nd
        nc.vector.tensor_copy(
            _strided_cols(y_nd, D, CW, NCH, 1),
            onecol.rearrange("p (u k) -> p u k", u=1).broadcast_to((128, NCH, 1)))

        with ExitStack() as outer_ctx:
            psG = outer_ctx.enter_context(tc.tile_pool(name="psG", bufs=1, space="PSUM"))
            pagg = psG.tile([32, CW], F32, name="pagg")

            stem_ctx = outer_ctx.enter_context(ExitStack())
            psB = stem_ctx.enter_context(tc.tile_pool(name="psB", bufs=3, space="PSUM"))
            psA = stem_ctx.enter_context(tc.tile_pool(name="psA", bufs=2, space="PSUM"))
            psK = stem_ctx.enter_context(tc.tile_pool(name="psK", bufs=2, space="PSUM"))

            maxt = sb.tile([128, NCH], BF16)
            sumt = sb.tile([128, NCH], F32)
            rcp = sb.tile([128, NCH], F32)
            rcpb = sb.tile([128, NCH], BF16)

            # HAM warm-up: dummy transposes with no DMA dependency keep the
            # PE clock gate at 8/8 until the first x piece lands.
            for i in range(20):
                pWr = psA.tile([128, 160], BF16, name="warm", tag="pA")
                nc.tensor.transpose(pWr[0:32, 128:160], identb[:], identb[:])
                if i < 12:
                    nc.tensor.transpose(pWr[:, 0:128], wmt[:], ident128[:])

            def emit_softmax(g0, gn):
                gs = slice(g0, g0 + gn)
                cs = slice(g0 * K, (g0 + gn) * K)
                lp3 = lnk[:, cs].rearrange("p (g k) -> p g k", g=gn)
                nc.vector.tensor_reduce(out=maxt[:, gs], in_=lp3,
                                        axis=mybir.AxisListType.X, op=ALU.max)
                mb = maxt[:, gs].rearrange("p (g u) -> p g u", u=1).broadcast_to((128, gn, K))
                nc.vector.tensor_tensor(
                    out=esub[:, cs].rearrange("p (g k) -> p g k", g=gn),
                    in0=lp3, in1=mb, op=ALU.subtract)
                nc.scalar.activation(e_sb[:, cs], esub[:, cs], AF.Exp)
                nc.vector.tensor_reduce(out=sumt[:, gs],
                                        in_=e_sb[:, cs].rearrange("p (g k) -> p g k", g=gn),
                                        axis=mybir.AxisListType.X, op=ALU.add)
                nc.vector.reciprocal(rcp[:, gs], sumt[:, gs])
                nc.vector.tensor_copy(rcpb[:, gs], rcp[:, gs])
                rb = rcpb[:, gs].rearrange("p (g u) -> p g u", u=1).broadcast_to((128, gn, K))
                nc.vector.tensor_tensor(out=a_sb[:, cs].rearrange("p (g k) -> p g k", g=gn),
                                        in0=e_sb[:, cs].rearrange("p (g k) -> p g k", g=gn),
                                        in1=rb, op=ALU.mult)
                # aggregation for the finished chunks, accumulated in PSUM
                for g in range(g0, g0 + gn):
                    nc.tensor.matmul(
                        pagg[:],
                        a_sb[:, g * K : (g + 1) * K],
                        y_nd[:, g * CW : (g + 1) * CW],
                        start=(g == 0), stop=(g == NCH - 1))

            for s in range(NS):
                ns = slice(s * 512, (s + 1) * 512)
                # --- stem B: y_dn[o, ns] = relu(sum_c wT[c,o]x[c,ns] + bias2[o])
                for o in range(NB):
                    pB = psB.tile([128, 512], F32)
                    for c in range(NB):
                        nc.tensor.matmul(
                            pB[:],
                            wT[:, c, o * 128 : (o + 1) * 128],
                            x_sb[:, c, ns],
                            start=(c == 0),
                            stop=(c == NB - 1),
                        )
                    dst = y_dn[:, o, ns]
                    if s % 2 == 0:
                        if has_bias2:
                            nc.scalar.activation(dst, pB[:], AF.Relu, bias=chv[:, o, 0:1])
                        else:
                            nc.scalar.activation(dst, pB[:], AF.Relu)
                    else:
                        if has_bias2:
                            nc.vector.tensor_scalar(
                                out=dst, in0=pB[:], scalar1=chv[:, o, 0:1],
                                scalar2=0.0, op0=ALU.add, op1=ALU.max)
                        else:
                            nc.vector.tensor_scalar_max(out=dst, in0=pB[:], scalar1=0.0)

                # --- stem A: y_nd chunk j via PE transpose of y_dn --------
                for half in range(2):
                    pA = psA.tile([128, 512], BF16)
                    j0 = 4 * s + 2 * half
                    for ci in range(2):
                        j = j0 + ci
                        jc = slice(j * 128, (j + 1) * 128)
                        for c in range(NB):
                            nc.tensor.transpose(
                                pA[:, (2 * ci + c) * 128 : (2 * ci + c + 1) * 128],
                                y_dn[:, c, jc], ident128[:])
                    dst = _strided_cols(y_nd, j0 * CW, CW, 2, D)
                    if half == 0:
                        nc.scalar.activation(dst, pA[:], AF.Identity)
                    else:
                        nc.vector.tensor_copy(dst, pA[:])

                # --- squares + logits; slice pairs through s=5, then
                #     per-slice for s=6,7 to shorten the serial tail --------
                if s in (1, 3, 5):
                    sls = (s - 1, s)
                elif s >= 6:
                    sls = (s,)
                else:
                    sls = ()
                if sls:
                    qs = slice(sls[0] * 512, (sls[-1] + 1) * 512)
                    if s >= 6:
                        nc.scalar.activation(ysq[:, 0, qs], y_dn[:, 0, qs], AF.Square)
                    else:
                        nc.gpsimd.tensor_mul(ysq[:, 0, qs], y_dn[:, 0, qs], y_dn[:, 0, qs])
                    nc.vector.tensor_tensor(out=ysq[:, 1, qs], in0=y_dn[:, 1, qs],
                                            in1=y_dn[:, 1, qs], op=ALU.mult)

                    for si, sl in enumerate(sls):
                        pK = psK.tile([32, 512], F32)
                        nsl = slice(sl * 512, (sl + 1) * 512)
                        nc.tensor.matmul(pK[:], ct2[:, 0, :], y_dn[:, 0, nsl],
                                         start=True, stop=False)
                        nc.tensor.matmul(pK[:], ct2[:, 1, :], y_dn[:, 1, nsl],
                                         start=False, stop=False)
                        nc.tensor.matmul(pK[:], srep[:], ysq[:, 0, nsl],
                                         start=False, stop=False)
                        nc.tensor.matmul(pK[:], srep[:], ysq[:, 1, nsl],
                                         start=False, stop=True)
                        dst = lkn[:, nsl]
                        if si == 0 and s != 7:
                            nc.scalar.activation(dst, pK[:], AF.Identity, bias=sc2col[:])
                        else:
                            nc.vector.tensor_scalar_add(out=dst, in0=pK[:],
                                                        scalar1=sc2col[:])
                    # logits (k,n) -> (n,k): big mid-phase batches on the
                    # DMA XBAR (1.25us fixed/instr, sync engine is free);
                    # the two tail batches on the PE (0.6us incl copy)
                    g0 = 4 * sls[0]
                    gt = 4 * len(sls)
                    if s < 6:
                        nc.sync.dma_start(
                            lnk[:, g0 * K : (g0 + gt) * K].rearrange("p (t k) -> p t k", t=gt),
                            lkn[:, sls[0] * 512 : (sls[-1] + 1) * 512], transpose=True)
                    else:
                        pL = psA.tile([128, 128], BF16, name="plog", tag="pA")
                        for j in range(4):
                            nc.tensor.transpose(
                                pL[:, j * K : (j + 1) * K],
                                lkn[:, (g0 + j) * 128 : (g0 + j + 1) * 128], identb[:])
                        nc.vector.tensor_copy(lnk[:, g0 * K : (g0 + 4) * K], pL[:])

                # --- softmax + agg over finished logits groups ------------
                if s == 3:
                    emit_softmax(0, 16)
                elif s == 5:
                    emit_softmax(16, 8)
                elif s == 6:
                    emit_softmax(24, 4)
                elif s == 7:
                    emit_softmax(28, 4)

            # ---- tail: rowsum fold, BN1 + mean, head, gate --------------
            stem_ctx.close()
            psT = outer_ctx.enter_context(tc.tile_pool(name="psT", bufs=2, space="PSUM"))

            # agg[k,d] = pagg[k,d] - rowsum_a[k] * centers[k,d]
            rsc = sb.tile([32, D], F32)
            nc.vector.tensor_scalar_mul(out=rsc[:], in0=ckd[:], scalar1=pagg[:, D : D + 1])
            agg_sb = sb.tile([32, D], F32)
            nc.vector.tensor_tensor(out=agg_sb[:], in0=pagg[:, 0:D], in1=rsc[:], op=ALU.subtract)

            # BN1 + relu + mean over k  ->  z per d-block (ACT accumulates)
            z_t = sb.tile([128, NB], F32)
            t_sb = sb.tile([128, NB, K], F32)
            for b in range(NB):
                pT = psT.tile([128, 32], F32)
                nc.tensor.transpose(pT[:], agg_sb[:, b * 128 : (b + 1) * 128], identf[:])
                nc.scalar.activation(t_sb[:, b, :], pT[:], AF.Relu,
                                     bias=chv[:, b, 2:3], scale=chv[:, b, 1:2],
                                     accum_out=z_t[:, b : b + 1])

            # head: gate = 1 + sigmoid(head_w @ z + head_b)
            zb = sb.tile([128, NB], BF16)
            nc.vector.tensor_copy(zb[:], z_t[:])
            gate = sb.tile([128, NB], F32)
            eg = sb.tile([128, NB], F32)
            for o in range(NB):
                pH = psT.tile([128, 1], F32)
                for c in range(NB):
                    nc.tensor.matmul(pH[:], hwT[:, c, o * 128 : (o + 1) * 128],
                                     zb[:, c : c + 1],
                                     start=(c == 0), stop=(c == NB - 1))
                nc.scalar.activation(eg[:, o : o + 1], pH[:], AF.Sigmoid,
                                     bias=chv[:, o, 3:4])
            nc.vector.tensor_scalar_add(out=gate[:], in0=eg[:], scalar1=1.0)

            # gating: out = relu(x * gate[d]) ; stream out per 512-col block
            for hh in range(8):
                cs = slice(hh * 512, (hh + 1) * 512)
                for o in range(NB):
                    if (2 * hh + o) % 2 == 0:
                        nc.vector.tensor_scalar(out=out_sb[:, o, cs], in0=x_sb[:, o, cs],
                                                scalar1=gate[:, o : o + 1], scalar2=0.0,
                                                op0=ALU.mult, op1=ALU.max)
                    else:
                        nc.scalar.activation(out_sb[:, o, cs], x_sb[:, o, cs],
                                             AF.Relu, scale=gate[:, o : o + 1])
                    qeng[o].dma_start(out_d[o * 128 : (o + 1) * 128, cs], out_sb[:, o, cs])

    nc.compile()
    return nc


_PROGRAM_CACHE = {}


def _get_program(has_bias2):
    key = bool(has_bias2)
    if key not in _PROGRAM_CACHE:
        _PROGRAM_CACHE[key] = _build_program(key)
    return _PROGRAM_CACHE[key]


def _host_params(conv_w, bn2_g, bn2_b, bn2_m, bn2_v, centers, scales,
                 bn1_g, bn1_b, bn1_m, bn1_v, head_w, head_b):
    scale2 = bn2_g / np.sqrt(bn2_v + EPS)
    wT = (conv_w * scale2[:, None]).T.astype(np.float32)             # (c, o)
    bias2 = (bn2_b - bn2_m * scale2).astype(np.float32)
    ct2 = (-2.0 * scales[None, :] * centers.T).astype(np.float32)    # (d, k)
    c2 = (centers * centers).sum(axis=1)
    scc = (scales * c2).astype(np.float32)                           # (k,)
    s1 = bn1_g / np.sqrt(bn1_v + EPS)
    bb1 = bn1_b - bn1_m * s1
    chv = np.stack([bias2, s1.astype(np.float32), bb1.astype(np.float32),
                    head_b.astype(np.float32)], axis=1).astype(np.float32)  # (d, 4)
    hwT = (head_w.T / np.float32(K)).astype(np.float32)              # (d, o)

    # pkb packed per c-block in SBUF row layout [128, c*PKB_COLS]
    pkb = np.zeros((128, NB * PKB_COLS), np.float32)
    for c in range(NB):
        base = c * PKB_COLS
        pkb[:, base + PKB_W : base + PKB_W + D] = wT[c * 128 : (c + 1) * 128, :]
        pkb[:, base + PKB_CT2 : base + PKB_CT2 + K] = ct2[c * 128 : (c + 1) * 128, :]
        pkb[:, base + PKB_ONE] = 1.0
        pkb[:, base + PKB_HW : base + PKB_HW + D] = hwT[c * 128 : (c + 1) * 128, :]

    import ml_dtypes
    pkb16 = pkb.astype(ml_dtypes.bfloat16)

    pkf = np.zeros((128, NB * 4), np.float32)
    for c in range(NB):
        pkf[:, c * 4 : (c + 1) * 4] = chv[c * 128 : (c + 1) * 128, :]

    pks = np.zeros((K, D + 2), np.float32)
    pks[:, 0:D] = centers
    pks[:, D] = scc
    srf = scales.reshape(1, K).astype(np.float16)
    return pkb16, pkf, pks, srf, bias2


def _ensure_profile_hook():
    """Register the axon NTFF profile hook if the image lacks antenv.axon_hooks."""
    import types

    if "antenv.axon_hooks" in sys.modules:
        return
    try:
        import antenv

        mod = types.ModuleType("antenv.axon_hooks")
        _hook = [None]
        mod.set_axon_ntff_profile_hook = lambda h: _hook.__setitem__(0, h)
        mod.get_axon_ntff_profile_hook = lambda: _hook[0]
        sys.modules["antenv.axon_hooks"] = mod
        antenv.axon_hooks = mod
        from trn_agent_boot.trn_boot import _ntff_profile_via_ctypes

        mod.set_axon_ntff_profile_hook(
            _ntff_profile_via_ctypes("/opt/axon/libaxon_pjrt.so"))
        import concourse.bass_utils as _bu

        _bu.upload_artifacts = lambda d: d  # no artifact store in this container
    except Exception as e:  # profiling is best-effort
        print(f"profile hook setup failed: {e}", file=sys.stderr)


def kernel(x, conv_w, bn2_g, bn2_b, bn2_m, bn2_v, centers, scales,
           bn1_g, bn1_b, bn1_m, bn1_v, head_w, head_b):
    import ml_dtypes

    x = np.asarray(x, dtype=np.float32)
    xh = x.astype(ml_dtypes.bfloat16)
    pkb16, pkf, pks, srf, bias2 = _host_params(
        np.asarray(conv_w, np.float32), np.asarray(bn2_g, np.float32),
        np.asarray(bn2_b, np.float32), np.asarray(bn2_m, np.float32),
        np.asarray(bn2_v, np.float32), np.asarray(centers, np.float32),
        np.asarray(scales, np.float32), np.asarray(bn1_g, np.float32),
        np.asarray(bn1_b, np.float32), np.asarray(bn1_m, np.float32),
        np.asarray(bn1_v, np.float32), np.asarray(head_w, np.float32),
        np.asarray(head_b, np.float32))
    has_bias2 = bool(np.abs(bias2).max() > 0)
    nc = _get_program(has_bias2)

    shared = {"pkb": pkb16, "pkf": pkf, "pks": pks, "srf": srf}
    in_maps = [dict(shared, x=np.ascontiguousarray(xh[b].reshape(D, HW)))
               for b in range(N_CORES)]

    trace = bool(int(os.environ.get("KERNEL_TRACE", "0")))
    kwargs = {}
    if trace:
        _ensure_profile_hook()
        tdir = os.environ.get("KERNEL_TRACE_DIR")
        if tdir:
            os.makedirs(tdir, exist_ok=True)
            kwargs["tmpdir"] = tdir
    res = run_bass_kernel_spmd(nc, in_maps, list(range(N_CORES)), trace=trace, **kwargs)
    if trace:
        kernel.last_exec_time_ns = res.exec_time_ns
        kernel.last_results = res
    out = np.stack([res.results[b]["out"].reshape(D, H, W) for b in range(N_CORES)])
    return out.astype(np.float32)
